# revision 1
# baseline (speedup 1.0000x reference)
"""Trainium2 Bass kernel: autoregressive GRU decoder (nn_Decoder).

B=1024, T=128, H=1024, I=128 (POSE=96 + TRAJ=32).
Data-parallel over batch across 8 NeuronCores (128 rows/core), no collectives.

Layout: fully transposed on-device — features on partitions, batch on the
free dim. h state kept as 8 K-tiles [128, 128]; x state [128, 128].
Matmul operands bf16, state fp32, PSUM accumulation fp32.

The pose/fc output head is folded into a single matmul:
tp = [[fc_p@lp_W + fc_h], [lp_W]] @ h' + btp, so y = x + tp in one shot.
"""

import sys

if "/opt/trn_rl_repo" not in sys.path:
    sys.path.insert(0, "/opt/trn_rl_repo")

import numpy as np
import ml_dtypes

B, T, H = 1024, 128, 1024
POSE, TRAJ = 96, 32
I = POSE + TRAJ  # 128
NCORES = 8
BL = B // NCORES  # 128 batch rows per core
KH = H // 128  # 8 h K-tiles
P = 128

# chunks (in units of 128-wide k-tiles) for the elementwise gate pipeline
_SC = [(0, 4), (4, 6), (6, 7), (7, 8)]
_CHUNK_OF = [0] * 4 + [1] * 2 + [2] + [3]

_BUILD_CACHE = {}
LAST_RESULTS = None


def _build(t_steps, reps=1, skeleton=False, pool_chain=False, bufs=2):
    """skeleton=True emits only the matmul stream (timing experiments).
    reps>1 wraps the step loop in For_i (skeleton only).
    pool_chain: run d/e/h'/cast on GpSimd (False -> DVE; HW-measured
    1.4us/step faster on DVE despite sim preferring GpSimd)."""
    import contextlib

    import concourse.bass as bass
    import concourse.tile as tile
    from concourse import bacc, mybir

    f32 = mybir.dt.float32
    bf16 = mybir.dt.bfloat16
    AF = mybir.ActivationFunctionType
    OP = mybir.AluOpType

    nc = bacc.Bacc(None, target_bir_lowering=False, debug=False)

    # ---- DRAM I/O ------------------------------------------------------
    dp = nc.declare_dram_parameter
    x0_d = dp("x0", [P, BL], f32, isOutput=False)             # x0^T
    h0_d = dp("h0", [P, KH, BL], f32, isOutput=False)         # h0^T k-tiles
    wrz_d = dp("wrz", [P, 9, 16, P], bf16, isOutput=False)    # [p,k,m,j] k0=x
    wnx_d = dp("wnx", [P, KH, P], bf16, isOutput=False)       # Win^T
    wnh_d = dp("wnh", [P, KH, KH, P], bf16, isOutput=False)   # Whn^T [p,k,m,j]
    wtp_d = dp("wtp", [P, KH, P], bf16, isOutput=False)       # tp weights^T
    brz_d = dp("brz", [P, 16], f32, isOutput=False)           # col m = bias m-tile
    bxn_d = dp("bxn", [P, KH], f32, isOutput=False)
    bhn_d = dp("bhn", [P, KH], f32, isOutput=False)
    btp_d = dp("btp", [P, 1], f32, isOutput=False)            # [lp_b; fc_b]
    yt_d = dp("yt", [t_steps, P, BL], f32, isOutput=True)     # y^T per step

    with tile.TileContext(nc) as tc:
        with (
            tc.tile_pool(name="const", bufs=1) as cpool,
            tc.tile_pool(name="state", bufs=bufs) as spool,
            tc.tile_pool(name="work", bufs=bufs) as wpool,
            tc.tile_pool(name="gates_ps", bufs=7, space="PSUM") as gpool,
            tc.tile_pool(name="tp_ps", bufs=1, space="PSUM") as tpool,
        ):
            # ---- one-time loads ----------------------------------------
            def load_const(dram, shape, dtype):
                t = cpool.tile(shape, dtype, tag=dram.name)
                nc.sync.dma_start(t[:], dram[:])
                return t

            wrz_s = load_const(wrz_d, [P, 9, 16, P], bf16)
            wnx_s = load_const(wnx_d, [P, KH, P], bf16)
            wnh_s = load_const(wnh_d, [P, KH, KH, P], bf16)
            wtp_s = load_const(wtp_d, [P, KH, P], bf16)
            brz_s = load_const(brz_d, [P, 16], f32)
            bxn_s = load_const(bxn_d, [P, KH], f32)
            bhn_s = load_const(bhn_d, [P, KH], f32)
            btp_s = load_const(btp_d, [P, 1], f32)

            h_f = [
                spool.tile([P, c1 - c0, BL], f32, tag=f"hf{i}", name=f"hf{i}")
                for i, (c0, c1) in enumerate(_SC)
            ]
            h_b = [
                spool.tile([P, c1 - c0, BL], bf16, tag=f"hb{i}", name=f"hb{i}")
                for i, (c0, c1) in enumerate(_SC)
            ]
            for i, (c0, c1) in enumerate(_SC):
                nc.sync.dma_start(h_f[i][:], h0_d[:, c0:c1, :])
                nc.vector.tensor_copy(h_b[i][:], h_f[i][:])
            x_f = spool.tile([P, BL], f32, tag="xf")
            nc.sync.dma_start(x_f[:], x0_d[:])
            x_b = spool.tile([P, BL], bf16, tag="xb")
            nc.vector.tensor_copy(x_b[:], x_f[:])

            def hbk(k):  # bf16 h k-tile accessor (chunked state tiles)
                i = _CHUNK_OF[k]
                return h_b[i][:, k - _SC[i][0], :]

            # ---- time steps --------------------------------------------
            HM = KH // 2  # m-tiles per 1-bank psum tile

            rep_ctx = (
                tc.For_i(0, reps, 1) if reps > 1 else contextlib.nullcontext()
            )
            with rep_ctx:
             for t in range(t_steps):
                 # One PSUM bank per tile ([128, 4, 128] fp32) so banks free
                 # individually.  m-tile m lives in (pair, m % 4).
                 ps_r = [
                     gpool.tile([P, 2, BL], f32, tag="ps", name=f"psr{i}_{t}")
                     for i in range(4)
                 ]
                 ps_hn = [
                     gpool.tile([P, 2, BL], f32, tag="ps", name=f"pshn{i}_{t}")
                     for i in range(4)
                 ]
                 ps_xn = [
                     gpool.tile([P, HM, BL], f32, tag="ps", name=f"psxn{i}_{t}")
                     for i in range(2)
                 ]
                 # z in 2-m-tile tiles: the tail sigmoids wait only on their
                 # own bank's matmuls instead of all of z.
                 _ZB = [(0, 2), (2, 4), (4, 6), (6, 7), (7, 8)]
                 ps_z = [
                     gpool.tile([P, z1 - z0, BL], f32, tag="ps",
                                name=f"psz{i}_{t}")
                     for i, (z0, z1) in enumerate(_ZB)
                 ]

                 def sl(pair, m):
                     return pair[m // HM][:, m % HM, :]

                 def slz(m):
                     for i, (z0, z1) in enumerate(_ZB):
                         if z0 <= m < z1:
                             return ps_z[i][:, m - z0, :]

                 def mm_r(m):
                     out = ps_r[m // 2][:, m % 2, :]
                     for k in range(KH):
                         nc.tensor.matmul(
                             out, wrz_s[:, 1 + k, m, :], hbk(k),
                             start=(k == 0), stop=False,
                         )
                     nc.tensor.matmul(
                         out, wrz_s[:, 0, m, :], x_b[:], start=False, stop=True
                     )

                 def mm_hn(m):
                     out = ps_hn[m // 2][:, m % 2, :]
                     for k in range(KH):
                         nc.tensor.matmul(
                             out, wnh_s[:, k, m, :], hbk(k),
                             start=(k == 0), stop=(k == KH - 1),
                         )

                 # PE emission order: r/hn pairs (chain-critical first), xn
                 # early (needs only x), z last (shallow post-chain).
                 mm_r(0); mm_hn(0); mm_r(1); mm_hn(1)
                 for m in range(KH):
                     nc.tensor.matmul(
                         sl(ps_xn, m), wnx_s[:, m, :], x_b[:],
                         start=True, stop=True,
                     )
                 for m in range(2, KH):
                     mm_r(m); mm_hn(m)
                 for m in range(KH):
                     out = slz(m)
                     for k in range(KH):
                         nc.tensor.matmul(
                             out, wrz_s[:, 1 + k, KH + m, :], hbk(k),
                             start=(k == 0), stop=False,
                         )
                     nc.tensor.matmul(
                         out, wrz_s[:, 0, KH + m, :], x_b[:],
                         start=False, stop=True,
                     )

                 if skeleton:
                     continue  # timing experiment: matmul stream only

                 # Chunked per-tile pipeline: every chunk tensor is its own
                 # tile so readers wait only on their chunk's writers.
                 r_s = [
                     wpool.tile([P, 2, BL], f32, tag=f"r{i}", name=f"r{i}_{t}")
                     for i in range(4)
                 ]
                 t1 = [
                     wpool.tile([P, 2, BL], f32, tag=f"t1{i}", name=f"t1{i}_{t}")
                     for i in range(4)
                 ]
                 t2c = [
                     wpool.tile([P, c1 - c0, BL], f32, tag=f"t2{i}",
                                name=f"t2{i}_{t}")
                     for i, (c0, c1) in enumerate(_SC)
                 ]
                 n_c = [
                     wpool.tile([P, c1 - c0, BL], f32, tag=f"n{i}",
                                name=f"n{i}_{t}")
                     for i, (c0, c1) in enumerate(_SC)
                 ]
                 d_c = [
                     wpool.tile([P, c1 - c0, BL], f32, tag=f"d{i}",
                                name=f"d{i}_{t}")
                     for i, (c0, c1) in enumerate(_SC)
                 ]
                 z_c = [
                     wpool.tile([P, c1 - c0, BL], f32, tag=f"z{i}",
                                name=f"z{i}_{t}")
                     for i, (c0, c1) in enumerate(_SC)
                 ]
                 e_c = [
                     wpool.tile([P, c1 - c0, BL], f32, tag=f"e{i}",
                                name=f"e{i}_{t}")
                     for i, (c0, c1) in enumerate(_SC)
                 ]
                 hf2 = [
                     spool.tile([P, c1 - c0, BL], f32, tag=f"hf{i}",
                                name=f"hf{i}_{t}")
                     for i, (c0, c1) in enumerate(_SC)
                 ]
                 hb2 = [
                     spool.tile([P, c1 - c0, BL], bf16, tag=f"hb{i}",
                                name=f"hb{i}_{t}")
                     for i, (c0, c1) in enumerate(_SC)
                 ]

                 def t2sl(m):
                     i = _CHUNK_OF[m]
                     return t2c[i][:, m - _SC[i][0], :]

                 def zsl(m):
                     i = _CHUNK_OF[m]
                     return z_c[i][:, m - _SC[i][0], :]

                 def sig_r(m):
                     nc.scalar.activation(
                         r_s[m // 2][:, m % 2, :], ps_r[m // 2][:, m % 2, :],
                         AF.Sigmoid, bias=brz_s[:, m : m + 1],
                     )

                 def t12(m):
                     nc.vector.scalar_tensor_tensor(
                         t1[m // 2][:, m % 2, :], ps_hn[m // 2][:, m % 2, :],
                         bhn_s[:, m : m + 1], r_s[m // 2][:, m % 2, :],
                         op0=OP.add, op1=OP.mult,
                     )
                     nc.vector.scalar_tensor_tensor(
                         t2sl(m), sl(ps_xn, m), bxn_s[:, m : m + 1],
                         t1[m // 2][:, m % 2, :], op0=OP.add, op1=OP.add,
                     )

                 def tanh_chunk(i):
                     nc.scalar.activation(n_c[i][:], t2c[i][:], AF.Tanh)

                 chain = nc.gpsimd if pool_chain else nc.vector

                 def d_chunk(i):
                     chain.tensor_sub(d_c[i][:], h_f[i][:], n_c[i][:])

                 def sig_z(m):
                     nc.scalar.activation(
                         zsl(m), slz(m), AF.Sigmoid,
                         bias=brz_s[:, KH + m : KH + m + 1],
                     )

                 def ehc_chunk(i, eng=None):
                     eng = eng or chain
                     eng.tensor_mul(e_c[i][:], z_c[i][:], d_c[i][:])
                     eng.tensor_add(hf2[i][:], n_c[i][:], e_c[i][:])
                     eng.tensor_copy(hb2[i][:], hf2[i][:])

                 # Emission interleave: per-engine order matches readiness
                 sig_r(0); sig_r(1); sig_r(2); sig_r(3)
                 t12(0); t12(1); t12(2); t12(3)
                 sig_r(4); sig_r(5)
                 t12(4); t12(5)
                 tanh_chunk(0)
                 sig_r(6); sig_r(7)
                 t12(6); t12(7)
                 tanh_chunk(1)
                 for m in range(4):
                     sig_z(m)
                 tanh_chunk(2); tanh_chunk(3)
                 for m in range(4, KH):
                     sig_z(m)

                 d_chunk(0); d_chunk(1)
                 ehc_chunk(0, nc.vector)
                 d_chunk(2); d_chunk(3)
                 ehc_chunk(1); ehc_chunk(2); ehc_chunk(3)

                 # tp = [[lp_W],[fc_p@lp_W + fc_h]] @ h_n  (one matmul set)
                 ps_tp_t = tpool.tile(
                     [P, HM, BL], f32, tag="tp", name=f"pstp_{t}"
                 )
                 ps_tp = ps_tp_t[:, 0, :]
                 for k in range(KH):
                     i = _CHUNK_OF[k]
                     nc.tensor.matmul(
                         ps_tp, wtp_s[:, k, :], hb2[i][:, k - _SC[i][0], :],
                         start=(k == 0), stop=(k == KH - 1),
                     )

                 # y = x + tp + btp ; y becomes x
                 x_f2 = spool.tile([P, BL], f32, tag="xf")
                 nc.vector.scalar_tensor_tensor(
                     x_f2[:], ps_tp, btp_s[:, 0:1], x_f[:],
                     op0=OP.add, op1=OP.add,
                 )
                 x_b2 = spool.tile([P, BL], bf16, tag="xb")
                 nc.vector.tensor_copy(x_b2[:], x_f2[:])
                 nc.sync.dma_start(yt_d[t, :, :], x_f2[:])

                 x_f, x_b, h_f, h_b = x_f2, x_b2, hf2, hb2

    nc.compile()
    return nc


def _prep_inputs(h, gt, Wih, Whh, bih, bhh, lp_W, lp_b, fc_W, fc_b):
    """Host-side: transpose into kernel layouts, cast weights to bf16."""
    bf = ml_dtypes.bfloat16
    f32 = np.float32

    # rz combined weights, transposed: [1152, 2048] -> [p, k(9), m(16), j]
    wrzT = np.concatenate([Wih[: 2 * H].T, Whh[: 2 * H].T], axis=0)
    wrz = np.empty((P, 9, 16, P), dtype=bf)
    for k in range(9):
        for m in range(16):
            wrz[:, k, m, :] = wrzT[k * P : (k + 1) * P, m * P : (m + 1) * P]

    wnxT = Wih[2 * H :].T  # [128, 1024]
    wnx = np.ascontiguousarray(wnxT.reshape(P, KH, P), dtype=bf)  # [p, m, j]

    wnhT = Whh[2 * H :].T  # [1024, 1024]
    wnh = np.empty((P, KH, KH, P), dtype=bf)
    for k in range(KH):
        for m in range(KH):
            wnh[:, k, m, :] = wnhT[k * P : (k + 1) * P, m * P : (m + 1) * P]

    # fold pose->traj head: traj = (fc_p@lp_W + fc_h)@h + (fc_p@lp_b + fc_b)
    fc_p = fc_W[:, :POSE].astype(np.float64)
    fc_h = fc_W[:, POSE:].astype(np.float64)
    m_traj = fc_p @ lp_W.astype(np.float64) + fc_h          # [32, 1024]
    m_tp = np.concatenate([m_traj, lp_W.astype(np.float64)], axis=0)  # [I, H]
    b_traj = fc_p @ lp_b.astype(np.float64) + fc_b          # [32]
    b_tp = np.concatenate([b_traj, lp_b.astype(np.float64)])  # [I]
    wtpT = m_tp.T  # [1024, 128]
    wtp = np.ascontiguousarray(
        wtpT.reshape(KH, P, P).transpose(1, 0, 2), dtype=bf
    )  # [p, k, m]

    b_rz = (bih + bhh)[: 2 * H].astype(f32)  # [2048]
    brz = np.ascontiguousarray(b_rz.reshape(16, P).T)  # [128, 16]
    bxn = np.ascontiguousarray(bih[2 * H :].reshape(KH, P).T.astype(f32))
    bhn = np.ascontiguousarray(bhh[2 * H :].reshape(KH, P).T.astype(f32))
    btp = b_tp.reshape(P, 1).astype(f32)

    shared = {
        "wrz": wrz, "wnx": wnx, "wnh": wnh, "wtp": wtp,
        "brz": brz, "bxn": bxn, "bhn": bhn, "btp": btp,
    }

    in_maps = []
    for c in range(NCORES):
        sl = slice(c * BL, (c + 1) * BL)
        x0 = np.ascontiguousarray(gt[sl, 0, :].T.astype(f32))  # [I, BL]
        h0 = np.ascontiguousarray(
            h[sl, :].T.reshape(KH, P, BL).transpose(1, 0, 2).astype(f32)
        )  # [p, k, b] = h[b, k*128+p]
        in_maps.append({"x0": x0, "h0": h0, **shared})
    return in_maps


def kernel(h, gt, Wih, Whh, bih, bhh, lp_W, lp_b, fc_W, fc_b, time_steps):
    from concourse.bass_utils import run_bass_kernel_spmd

    t_steps = int(time_steps)

    h = np.asarray(h, np.float32)
    gt = np.asarray(gt, np.float32)

    if t_steps not in _BUILD_CACHE:
        _BUILD_CACHE[t_steps] = _build(t_steps)
    nc = _BUILD_CACHE[t_steps]

    in_maps = _prep_inputs(
        h, gt, np.asarray(Wih, np.float32), np.asarray(Whh, np.float32),
        np.asarray(bih, np.float32), np.asarray(bhh, np.float32),
        np.asarray(lp_W, np.float32), np.asarray(lp_b, np.float32),
        np.asarray(fc_W, np.float32), np.asarray(fc_b, np.float32),
    )

    import os

    trace = bool(os.environ.get("KERNEL_TRACE"))
    res = run_bass_kernel_spmd(
        nc, in_maps, core_ids=list(range(NCORES)), trace=trace
    )
    global LAST_RESULTS
    LAST_RESULTS = res

    out = np.empty((B, t_steps, I), dtype=np.float32)
    for c in range(NCORES):
        yt = res.results[c]["yt"]  # [T, I_k, BL]
        out[c * BL : (c + 1) * BL] = yt.transpose(2, 0, 1)
    return out

